# revision 21
# baseline (speedup 1.0000x reference)
"""Causal dot-product attention (B=4, H=16, S=2048, D=128) on 8 TRN2 NeuronCores.

Sharding: batch*heads = 64 (b,h) pairs -> 8 heads per core (head parallel, no
communication). Each core runs a flash-attention-style kernel:

  - Host pre-transposes Q,K per head to [D=128, S] (fp32) so both matmul
    operands have the contraction dim (D) on partitions, and packs V per head
    as [kpos=128, kblock, D+1] in bf16 with a ones column at d=128 (the PV
    matmul then produces the softmax denominator for free).
  - Device computes S^T blocks st[k, q] = K^T.T @ Q^T via float32r matmuls
    (moving dim 256 -> full PE rate), exp(scale*st) on the scalar engine
    (PSUM -> bf16 SBUF), a triangular-mask multiply on diagonal blocks only
    (DVE), then PV: out[q, 0:129] += pt_block.T @ V_aug in bf16, accumulated
    over k-blocks in PSUM. Block-causal skipping halves the work.
  - Normalize: out = acc[:, :128] * reciprocal(acc[:, 128]) on DVE, DMA out.

No max-subtraction is needed: scores are ~N(0,1) after the 1/sqrt(128) scale,
so exp() stays in [e-6, e+6] comfortably inside fp32/bf16 range.
"""

import math
import sys
from contextlib import ExitStack

import numpy as np

for _p in ("/opt/trn_rl_repo", "/root/.axon_site/_ro/trn_rl_repo"):
    if _p not in sys.path:
        sys.path.append(_p)

import ml_dtypes

import concourse.bass as bass
import concourse.tile as tile
from concourse import bacc, mybir
from concourse.bass_utils import run_bass_kernel_spmd

F32 = mybir.dt.float32
F32R = mybir.dt.float32r
BF16 = mybir.dt.bfloat16
AF = mybir.ActivationFunctionType

# Problem constants (hardcoded; kernel.py must be self-contained).
B, H, S, D = 4, 16, 2048, 128
P = 128
N_CORES = 8
NH = (B * H) // N_CORES  # heads per core = 8
SCALE = 1.0 / math.sqrt(128.0)  # D_MODEL = 128

QTW = 256  # q-tile width (matmul moving dim; >=256 keeps float32r at full rate)
GROUP = 4  # k-chunks per PSUM st tile (2 banks) / per exp() instruction


def build_nc(nh=NH, s=S, qk_dtype=F32R, pv_dtype=BF16):
    nkb = s // P  # k-blocks per head
    nqt = s // QTW  # q-tiles per head
    sub = QTW // P  # q-subtiles (of 128) per q-tile

    nc = bacc.Bacc("TRN2", target_bir_lowering=False, debug=False,
                   enable_asserts=False)
    qt_d = nc.declare_dram_parameter("qt", [nh, P, s], qk_dtype,
                                     isOutput=False).ap()
    kt_d = nc.declare_dram_parameter("kt", [nh, P, s], qk_dtype,
                                     isOutput=False).ap()
    v_d = nc.declare_dram_parameter("v", [nh, P, nkb, D + 1], BF16,
                                    isOutput=False).ap()
    mask_d = nc.declare_dram_parameter("mask", [P, P], BF16, isOutput=False).ap()
    out_d = nc.declare_dram_parameter("out", [nh, s, D], F32, isOutput=True).ap()

    with tile.TileContext(nc) as tc, ExitStack() as ctx:
        kt_pool = ctx.enter_context(tc.tile_pool(name="kt_pool", bufs=2))
        v_pool = ctx.enter_context(tc.tile_pool(name="v_pool", bufs=2))
        qt_pool = ctx.enter_context(tc.tile_pool(name="qt_pool", bufs=4))
        pt_pool = ctx.enter_context(tc.tile_pool(name="pt_pool", bufs=4))
        st_pool = ctx.enter_context(tc.tile_pool(name="st_pool", bufs=2,
                                                 space="PSUM"))
        acc_pool = ctx.enter_context(tc.tile_pool(name="acc_pool", bufs=4,
                                                  space="PSUM"))
        out_pool = ctx.enter_context(tc.tile_pool(name="out_pool", bufs=4))
        rl_pool = ctx.enter_context(tc.tile_pool(name="rl_pool", bufs=4))
        misc = ctx.enter_context(tc.tile_pool(name="misc", bufs=1))

        mask_t = misc.tile([P, P], BF16)
        nc.sync.dma_start(out=mask_t[:], in_=mask_d)

        # Streaming state: st/pt tiles fill with up to GROUP k-chunks before a
        # single exp() drains them; the stream runs across q-tile boundaries.
        # PV consumption of a group is deferred TWO groups: when PV(g) reaches
        # the PE queue head, its dependency exp(g) finished two ACT-periods
        # ago, so the in-order PE queue never head-of-line blocks ready QK
        # work behind a PV that waits on the in-flight exp.
        state = {"st": None, "pt": None, "fill": 0, "entries": [],
                 "pending": []}

        def normalize(h, i, acc_t):
            for sI in range(sub):
                g = i * sub + sI
                rl = rl_pool.tile([P, 1], F32, tag="rl", name="rl")
                nc.vector.reciprocal(rl[:], acc_t[:, sI * 129 + 128:sI * 129 + 129])
                o_t = out_pool.tile([P, D], F32, tag="o", name="o_t")
                nc.vector.tensor_scalar_mul(o_t[:], acc_t[:, sI * 129:sI * 129 + D],
                                            rl[:])
                # Output stores go on the (otherwise idle) GpSimd queue so
                # their normalize-waits never head-of-line block the sync
                # queue that prefetches the next head's K/V/Q.
                nc.gpsimd.dma_start(out=out_d[h, g * P:(g + 1) * P, :], in_=o_t[:])

        def emit_pv(group):
            pt_t, entries, v_t = group
            for (pos, eh, i, j, acc_t) in entries:
                off = pos * QTW
                for sI in range(sub):
                    g = i * sub + sI  # global q-block index
                    if j > g:
                        continue  # fully-masked block: skip PV entirely
                    ps = pt_t[:, off + sI * P: off + (sI + 1) * P]
                    if j == g:
                        nc.vector.tensor_mul(ps, ps, mask_t[:])
                    # One PSUM accumulation group per acc bank: start=True arms
                    # the whole 2KB zero region, so only the first matmul into
                    # the tile starts and only the last one stops.
                    nc.tensor.matmul(acc_t[:, sI * 129:(sI + 1) * 129],
                                     lhsT=ps, rhs=v_t[:, j],
                                     start=(j == 0 and sI == 0),
                                     stop=(sI == sub - 1 and j == i * sub + sub - 1))
            for (pos, eh, i, j, acc_t) in entries:
                if j == (i + 1) * sub - 1:
                    normalize(eh, i, acc_t)

        def flush(v_t, final=False):
            pend = state["pending"]
            if state["fill"]:
                w = state["fill"] * QTW
                st_t, pt_t = state["st"], state["pt"]
                nc.scalar.activation(pt_t[:, :w], st_t[:, :w], AF.Exp,
                                     bias=0.0, scale=SCALE)
                pend.append((pt_t, state["entries"], v_t))
            lag = 0 if final else 2
            while len(pend) > lag:
                emit_pv(pend.pop(0))
            state.update(st=None, pt=None, fill=0, entries=[], pending=pend)

        for h in range(nh):
            # Load order matters: the sync queue issues ~1 DMA per 620ns, so
            # the first-needed data (kt chunk 0, first qt) goes first. V rides
            # the DVE queue so it never contends with the kt/qt stream.
            kt_t = kt_pool.tile([P, s], qk_dtype, tag="kt", name="kt_t")
            nchunk = max(1, s // 256) if h == 0 else max(1, s // 512)
            w = s // nchunk
            nc.sync.dma_start(out=kt_t[:, :w], in_=kt_d[h, :, :w])
            v_t = v_pool.tile([P, nkb, D + 1], pv_dtype, tag="v", name="v_t")
            vchunk = max(1, nkb // 4)

            for i in range(nqt):
                qt_t = qt_pool.tile([P, QTW], qk_dtype, tag="qt", name="qt_t")
                if h == 0 and i == 0:
                    nc.sync.dma_start(out=qt_t[:, :QTW // 2],
                                      in_=qt_d[h, :, :QTW // 2])
                    nc.sync.dma_start(out=qt_t[:, QTW // 2:],
                                      in_=qt_d[h, :, QTW // 2:QTW])
                else:
                    nc.sync.dma_start(out=qt_t[:],
                                      in_=qt_d[h, :, i * QTW:(i + 1) * QTW])
                if i == 0:
                    for c in range(0, nkb, vchunk):
                        nc.gpsimd.dma_start(out=v_t[:, c:c + vchunk],
                                            in_=v_d[h, :, c:c + vchunk])
                elif i == 1:
                    for c in range(1, nchunk):
                        nc.sync.dma_start(out=kt_t[:, c * w:(c + 1) * w],
                                          in_=kt_d[h, :, c * w:(c + 1) * w])
                acc_t = acc_pool.tile([P, sub * 129], F32, tag="acc", name="acc_t")
                for j in range((i + 1) * sub):  # causal k-blocks only
                    if state["fill"] == 0:
                        state["st"] = st_pool.tile([P, GROUP * QTW], F32,
                                                   tag="st", name="st_t")
                        state["pt"] = pt_pool.tile([P, GROUP * QTW], pv_dtype,
                                                   tag="pt", name="pt_t")
                    pos = state["fill"]
                    nc.tensor.matmul(state["st"][:, pos * QTW:(pos + 1) * QTW],
                                     lhsT=kt_t[:, j * P:(j + 1) * P], rhs=qt_t[:],
                                     start=True, stop=True)
                    state["entries"].append((pos, h, i, j, acc_t))
                    state["fill"] += 1
                    if state["fill"] == GROUP:
                        flush(v_t)
        flush(None, final=True)
    nc.compile()
    return nc


_NC = None


def _get_nc():
    global _NC
    if _NC is None:
        _NC = build_nc()
    return _NC


def prepare_in_maps(Q, K, V):
    """Shard + lay out full [B,H,S,D] inputs into per-core in_maps."""
    Qf = np.ascontiguousarray(np.asarray(Q, dtype=np.float32)).reshape(B * H, S, D)
    Kf = np.ascontiguousarray(np.asarray(K, dtype=np.float32)).reshape(B * H, S, D)
    Vf = np.ascontiguousarray(np.asarray(V, dtype=np.float32)).reshape(B * H, S, D)
    nkb = S // P
    mask = np.triu(np.ones((P, P), dtype=np.float32)).astype(ml_dtypes.bfloat16)
    in_maps = []
    for c in range(N_CORES):
        hs = slice(c * NH, (c + 1) * NH)
        qt = np.ascontiguousarray(Qf[hs].transpose(0, 2, 1))  # [NH, D, S]
        kt = np.ascontiguousarray(Kf[hs].transpose(0, 2, 1))  # [NH, D, S]
        # V: [NH, S, D] -> [NH, kblock, kpos, D] -> [NH, kpos, kblock, D]
        vv = Vf[hs].reshape(NH, nkb, P, D).transpose(0, 2, 1, 3)
        v_aug = np.ones((NH, P, nkb, D + 1), dtype=ml_dtypes.bfloat16)
        v_aug[..., :D] = vv.astype(ml_dtypes.bfloat16)
        in_maps.append({"qt": qt, "kt": kt, "v": v_aug, "mask": mask})
    return in_maps


def gather_out(results):
    out = np.concatenate([np.asarray(r["out"], dtype=np.float32)
                          for r in results], axis=0)  # [64, S, D]
    return out.reshape(B, H, S, D)


def kernel(Q, K, V):
    in_maps = prepare_in_maps(Q, K, V)
    nc = _get_nc()
    res = run_bass_kernel_spmd(nc, in_maps, core_ids=list(range(N_CORES)))
    return gather_out(res.results)
